# revision 1
# baseline (speedup 1.0000x reference)
"""v7: v5/v6 + software-pipelined prep (next diagram's prep emitted mid-loop),
job-interleaved prep ordering, split input DMA."""

import numpy as np
from contextlib import ExitStack

import concourse.bass as bass
import concourse.bacc as bacc
import concourse.tile as tile
from concourse import mybir

F32 = mybir.dt.float32

RESOLUTION = 50
SIGMA = 0.05
NF = float(np.float32(1.0 / (2.0 * SIGMA**2 + 1e-8)))
XLO, XHI = -0.06, 1.06


def make_host_constants(Nc=30, njobs=3):
    JC = Nc // njobs
    x = np.linspace(0.0, 1.0, RESOLUTION).astype(np.float32).astype(np.float64)
    xc = np.linspace(XLO, XHI, Nc)
    jobs = np.array_split(np.arange(Nc), njobs)
    bs = np.linspace(0.0, 1.0, 4001)
    Phi = np.exp(-NF * (xc[None, :] - bs[:, None]) ** 2)
    G = np.exp(-NF * (x[None, :] - bs[:, None]) ** 2)
    W = np.linalg.solve(Phi.T @ Phi + 1e-10 * np.eye(Nc), Phi.T @ G)
    u400 = np.zeros(Nc)
    centers = np.zeros(njobs)
    kappa = np.zeros(Nc)
    for ji, J in enumerate(jobs):
        cJ = 0.5 * (xc[J[0]] + xc[J[-1]])
        centers[ji] = cJ
        u = xc[J] - cJ
        u400[J] = 2.0 * NF * u
        k = np.zeros(len(J))
        for t in range(2):
            k[2 + t] = k[t] + NF * (u[2 + t] ** 2 - u[t] ** 2)
        for t in range(4):
            k[4 + t] = k[t] + NF * (u[4 + t] ** 2 - u[t] ** 2)
        if len(J) == 10:
            for t in range(2):
                k[8 + t] = k[4 + t] + NF * (u[8 + t] ** 2 - u[4 + t] ** 2)
        kappa[J] = k
    Wt = W * np.exp(-kappa)[:, None]
    return u400.astype(np.float32), centers.astype(np.float32), Wt.astype(np.float32)


def build_kernel(DG=4, N=65536, Nc=30, njobs=3, G=128, debug=False):
    assert N % 128 == 0
    CH = N // 128
    assert CH % G == 0
    ngroups = CH // G
    JC = Nc // njobs
    assert JC in (8, 10)

    u400, centers, Wt = make_host_constants(Nc, njobs)
    h400 = float(u400[1] - u400[0])
    SQNF = float(np.float32(np.sqrt(NF)))

    nc = bacc.Bacc("TRN2", target_bir_lowering=False, debug=debug)

    diagrams = nc.declare_dram_parameter("diagrams", [DG, N, 2], F32, isOutput=False)
    wtx_d = nc.declare_dram_parameter("wtx", [Nc, RESOLUTION], F32, isOutput=False)
    wty_d = nc.declare_dram_parameter("wty", [Nc, RESOLUTION], F32, isOutput=False)
    out_d = nc.declare_dram_parameter("out", [DG, RESOLUTION, RESOLUTION], F32, isOutput=True)

    with ExitStack() as ctx:
        tc = ctx.enter_context(tile.TileContext(nc))
        singles = ctx.enter_context(tc.tile_pool(name="singles", bufs=1))
        raws = ctx.enter_context(tc.tile_pool(name="raws", bufs=2))
        preps = ctx.enter_context(tc.tile_pool(name="preps", bufs=2))
        tmps = ctx.enter_context(tc.tile_pool(name="tmps", bufs=2))
        bigs = ctx.enter_context(tc.tile_pool(name="bigs", bufs=2))
        psums = ctx.enter_context(tc.tile_pool(name="psums", bufs=2, space="PSUM"))
        outs = ctx.enter_context(tc.tile_pool(name="outs", bufs=2))

        bias_t = {}
        for ji in range(njobs):
            cJ = float(centers[ji])
            vals = {
                "sq0": -SQNF * cJ - float(u400[ji * JC + 0]) / (2.0 * SQNF),
                "sq1": -SQNF * cJ - float(u400[ji * JC + 1]) / (2.0 * SQNF),
                "r2": -2 * h400 * cJ,
                "r4": -4 * h400 * cJ,
            }
            for key, v in vals.items():
                bt = singles.tile([128, 1], F32, tag=f"bias{ji}_{key}",
                                  name=f"bias{ji}_{key}")
                nc.vector.memset(bt[:], float(v))
                bias_t[(ji, key)] = bt

        wtx_t = singles.tile([Nc, RESOLUTION], F32)
        nc.sync.dma_start(out=wtx_t[:], in_=wtx_d[:])
        wty_t = singles.tile([Nc, RESOLUTION], F32)
        nc.sync.dma_start(out=wty_t[:], in_=wty_d[:])

        def emit_prep_start(dg):
            raw = raws.tile([128, CH * 2], F32, tag="raw", name=f"raw{dg}")
            dsrc = diagrams[dg].rearrange("(p c) t -> p (c t)", p=128)
            for si, eng in enumerate((nc.sync, nc.scalar, nc.sync, nc.scalar)):
                sl = slice(si * CH * 2 // 4, (si + 1) * CH * 2 // 4)
                eng.dma_start(out=raw[:, sl], in_=dsrc[:, sl])
            raw3 = raw.rearrange("p (c t) -> p c t", t=2)
            b_ap = raw3[:, :, 0]
            d_ap = raw3[:, :, 1]

            pw = tmps.tile([128, CH], F32, tag="pw", name=f"pw{dg}")
            nc.vector.tensor_sub(pw[:], d_ap, b_ap)
            w_t = preps.tile([128, CH], F32, tag="w", name=f"w{dg}")
            nc.scalar.activation(
                out=w_t[:], in_=pw[:],
                func=mybir.ActivationFunctionType.Square, scale=1.0,
            )

            t = {"w": w_t, "b_ap": b_ap, "d_ap": d_ap}
            for ax in ("x", "y"):
                for nm in ("es0", "es1", "r2", "r4"):
                    t[nm + ax] = preps.tile([128, njobs, CH], F32,
                                            tag=f"{nm}{ax}", name=f"{nm}{ax}{dg}")
            return t

        def emit_prep_job(t, ji):
            # center-subtract folded into ACT bias; reads raw strided b/d
            for ax, src in (("x", t["b_ap"]), ("y", t["d_ap"])):
                for nm, key in (("es0", "sq0"), ("es1", "sq1")):
                    nc.scalar.activation(
                        out=t[nm + ax][:, ji], in_=src,
                        func=mybir.ActivationFunctionType.Square,
                        scale=SQNF, bias=bias_t[(ji, key)][:],
                    )
                for nm, s in (("r2", 2 * h400), ("r4", 4 * h400)):
                    nc.scalar.activation(
                        out=t[nm + ax][:, ji], in_=src,
                        func=mybir.ActivationFunctionType.Exp, scale=float(s),
                        bias=bias_t[(ji, nm)][:],
                    )

        def emit_groups(dg, t, hp, glo, ghi, job_prep_cb=None):
            for g in range(glo, ghi):
                c0 = g * G
                T = {}
                for ax in ("x", "y"):
                    tg = bigs.tile([128, G, Nc], F32, tag="T", bufs=5, name=f"T{ax}{dg}_{g}")
                    for ji in range(njobs):
                        if job_prep_cb is not None and ax == "x":
                            job_prep_cb(ji)
                        j0 = ji * JC
                        nc.scalar.activation(
                            out=tg[:, :, j0], in_=t["es0" + ax][:, ji, c0:c0 + G],
                            func=mybir.ActivationFunctionType.Exp, scale=-1.0,
                        )
                        nc.scalar.activation(
                            out=tg[:, :, j0 + 1], in_=t["es1" + ax][:, ji, c0:c0 + G],
                            func=mybir.ActivationFunctionType.Exp, scale=-1.0,
                        )
                        if ax == "x":
                            wv = t["w"][:, c0:c0 + G]
                            in1 = bass.AP(tensor=wv.tensor, offset=wv.offset,
                                          ap=[wv.ap[0], wv.ap[1], [0, 2]])
                            nc.vector.tensor_mul(
                                tg[:, :, j0:j0 + 2], tg[:, :, j0:j0 + 2], in1
                            )
                        r2v = t["r2" + ax][:, ji, c0:c0 + G]
                        in1 = bass.AP(tensor=r2v.tensor, offset=r2v.offset,
                                      ap=[r2v.ap[0], r2v.ap[1], [0, 2]])
                        nc.vector.tensor_mul(
                            tg[:, :, j0 + 2:j0 + 4], tg[:, :, j0:j0 + 2], in1
                        )
                        r4v = t["r4" + ax][:, ji, c0:c0 + G]
                        in1 = bass.AP(tensor=r4v.tensor, offset=r4v.offset,
                                      ap=[r4v.ap[0], r4v.ap[1], [0, 4]])
                        nc.vector.tensor_mul(
                            tg[:, :, j0 + 4:j0 + 8], tg[:, :, j0:j0 + 4], in1
                        )
                        if JC == 10:
                            in1b = bass.AP(tensor=r4v.tensor, offset=r4v.offset,
                                           ap=[r4v.ap[0], r4v.ap[1], [0, 2]])
                            nc.vector.tensor_mul(
                                tg[:, :, j0 + 8:j0 + 10], tg[:, :, j0 + 4:j0 + 6], in1b
                            )
                    T[ax] = tg
                for c in range(G):
                    q = c % 4
                    cg = c0 + c
                    nc.tensor.matmul(
                        hp[32 * q:32 * q + Nc, :Nc],
                        T["x"][:, c], T["y"][:, c],
                        start=(cg == q), stop=(cg == CH - 4 + q),
                        tile_position=(0, 32 * q),
                        skip_group_check=True,
                    )

        def emit_tail(dg, hp):
            hs = outs.tile([Nc, Nc], F32, tag="hs", name=f"hs{dg}")
            nc.vector.tensor_copy(hs[:], hp[0:Nc, :Nc])
            for q in range(1, 4):
                nc.vector.tensor_add(hs[:], hs[:], hp[32 * q:32 * q + Nc, :Nc])
            p1 = psums.tile([Nc, RESOLUTION], F32, tag="p1", name=f"p1{dg}")
            nc.tensor.matmul(p1[:], hs[:], wtx_t[:], start=True, stop=True)
            o1 = outs.tile([Nc, RESOLUTION], F32, tag="o1", name=f"o1{dg}")
            nc.vector.tensor_copy(o1[:], p1[:])
            p2 = psums.tile([RESOLUTION, RESOLUTION], F32, tag="p2", name=f"p2{dg}")
            nc.tensor.matmul(p2[:], o1[:], wty_t[:], start=True, stop=True)
            o2 = outs.tile([RESOLUTION, RESOLUTION], F32, tag="o2", name=f"o2{dg}")
            nc.vector.tensor_copy(o2[:], p2[:])
            nc.sync.dma_start(out=out_d[dg], in_=o2[:])

        # software pipeline: prep(dg+1) jobs spread between dg's groups;
        # diagram 0's prep interleaved into its first group's build
        t = emit_prep_start(0)
        for dg in range(DG):
            hp = psums.tile([128, 32], F32, tag="H", name=f"H{dg}")
            t_next = None
            for g in range(ngroups):
                cb = (lambda ji: emit_prep_job(t, ji)) if (dg == 0 and g == 0) else None
                emit_groups(dg, t, hp, g, g + 1, job_prep_cb=cb)
                if dg + 1 < DG:
                    if g == 0:
                        t_next = emit_prep_start(dg + 1)
                        emit_prep_job(t_next, 0)
                    elif g - 1 < njobs - 1:
                        emit_prep_job(t_next, g)
            emit_tail(dg, hp)
            t = t_next

    nc.compile()
    return nc, {"wtx": Wt.copy(), "wty": Wt.copy()}



_CACHE = {}


def _get_built():
    if "k" not in _CACHE:
        _CACHE["k"] = build_kernel(DG=4, N=65536, Nc=30, njobs=3, G=128)
    return _CACHE["k"]


def kernel(diagrams):
    """Full-input entry point: diagrams [32, 65536, 2] fp32 -> [32, 50, 50] fp32.

    Shards the batch axis over 8 NeuronCores (4 diagrams each), runs the
    Bass kernel SPMD, gathers per-core outputs.
    """
    from concourse.bass_utils import run_bass_kernel_spmd

    diagrams = np.ascontiguousarray(np.asarray(diagrams, dtype=np.float32))
    B, N, two = diagrams.shape
    assert (B, N, two) == (32, 65536, 2), (B, N, two)
    nc, consts = _get_built()
    in_maps = []
    for core in range(8):
        m = {"diagrams": diagrams[core * 4:(core + 1) * 4]}
        m.update(consts)
        in_maps.append(m)
    res = run_bass_kernel_spmd(nc, in_maps, core_ids=list(range(8)))
    out = np.concatenate([res.results[c]["out"] for c in range(8)], axis=0)
    return out.astype(np.float32)



# revision 12
# speedup vs baseline: 1.9163x; 1.9163x over previous
"""v8: bf16 axis-paired coefficient build.

Key changes vs v7:
- Nc=20 basis (njobs=2, JC=10) instead of 30; accuracy traded within the 2e-2
  gate (host-simulated end-to-end max-rel err ~6e-3 incl. bf16 rounding).
- T layout [128, CH, Nc, 2(axis)]: every DVE op has a packed 2-byte last dim
  -> 2x/4x DVE modes; x and y processed together in each instruction.
- Seeds via ONE Derivative_Erf activation each (= 2/sqrt(pi) * exp(-x^2)),
  phase-grouped so the act table switches only twice.
- Sequential r2 chain (col k = col k-2 * r2), no r4 -> no overflow, fewer ACT
  passes. Per-job centering keeps intermediates finite in bf16.
- Weight fold: |d-b| multiplied into BOTH axis seeds (product = persistence^2).
- bf16 matmuls batched 4 channels/instruction (out [80,80], cross blocks
  unread garbage) -> 128 matmul instrs per diagram at 1 cy/row.
- W fitted on host against the bf16-simulated chain basis (absorbs table
  constant 2/sqrt(pi), kappa factors, and systematic rounding bias).
"""

import numpy as np
from contextlib import ExitStack

import concourse.bass as bass
import concourse.bacc as bacc
import concourse.tile as tile
from concourse import mybir

F32 = mybir.dt.float32
BF16 = mybir.dt.bfloat16

RESOLUTION = 50
SIGMA = 0.05
NF = float(np.float32(1.0 / (2.0 * SIGMA**2 + 1e-8)))
SQNF = float(np.float32(np.sqrt(NF)))
XLO, XHI = -0.06, 1.06


def _bf16(x):
    x = np.asarray(x, np.float32)
    u = x.view(np.uint32)
    r = ((u >> 16) & 1).astype(np.uint32)
    return ((u + 0x7FFF + r) & 0xFFFF0000).view(np.float32)


def make_host_constants(Nc=20, njobs=2, ridge=1e-6, nv=16001):
    """Fit W against the bf16-simulated chain basis on a dense grid."""
    JC = Nc // njobs
    xc = np.linspace(XLO, XHI, Nc)
    h = float(xc[1] - xc[0])
    v = np.linspace(0.0, 1.0, nv).astype(np.float32)
    Phi = np.zeros((nv, Nc), np.float32)
    for j in range(njobs):
        J = np.arange(j * JC, (j + 1) * JC)
        cJ = 0.5 * (xc[J[0]] + xc[J[-1]])
        for s in range(2):
            arg = (np.float32(SQNF) * v + np.float32(-SQNF * xc[J[s]])).astype(np.float64)
            Phi[:, j * JC + s] = _bf16((2 / np.sqrt(np.pi)) * np.exp(-arg ** 2))
        r2 = _bf16(np.exp(4 * NF * h * (v.astype(np.float64) - cJ)))
        for k in range(2, JC):
            Phi[:, j * JC + k] = _bf16(Phi[:, j * JC + k - 2] * r2)
    x = np.linspace(0, 1, RESOLUTION)
    G = np.exp(-NF * (x[None, :] - v[:, None].astype(np.float64)) ** 2)
    P = Phi.astype(np.float64)
    A = P.T @ P
    W = np.linalg.solve(A + ridge * np.diag(np.diag(A)), P.T @ G)
    centers = np.array([0.5 * (xc[j * JC] + xc[(j + 1) * JC - 1]) for j in range(njobs)])
    return W.astype(np.float32), xc, h, centers


def build_kernel(DG=4, N=65536, Nc=20, njobs=2, debug=False):
    assert N % 128 == 0
    CH = N // 128
    JC = Nc // njobs
    KB = 4            # channels batched per matmul
    NB = CH // KB     # matmul instructions per diagram
    W, xc, h, centers = make_host_constants(Nc, njobs)

    nc = bacc.Bacc("TRN2", target_bir_lowering=False, debug=debug)

    diagrams = nc.declare_dram_parameter("diagrams", [DG, N, 2], F32, isOutput=False)
    wtx_d = nc.declare_dram_parameter("wtx", [Nc, RESOLUTION], F32, isOutput=False)
    wty_d = nc.declare_dram_parameter("wty", [Nc, RESOLUTION], F32, isOutput=False)
    out_d = nc.declare_dram_parameter("out", [DG, RESOLUTION, RESOLUTION], F32, isOutput=True)
    sel_d = nc.declare_dram_parameter("sel", [KB, KB * Nc, Nc], F32, isOutput=False)

    DERF = mybir.ActivationFunctionType.Derivative_Erf
    EXP = mybir.ActivationFunctionType.Exp
    ABS = mybir.ActivationFunctionType.Abs

    def bcast(ap_obj, n, pos):
        """Insert a [0, n] broadcast dim at position pos of an AP's dims."""
        dims = list(ap_obj.ap)
        dims.insert(pos, [0, n])
        return bass.AP(tensor=ap_obj.tensor, offset=ap_obj.offset, ap=dims)

    with ExitStack() as ctx:
        tc = ctx.enter_context(tile.TileContext(nc))
        singles = ctx.enter_context(tc.tile_pool(name="singles", bufs=1))
        raws = ctx.enter_context(tc.tile_pool(name="raws", bufs=4))
        seeds = ctx.enter_context(tc.tile_pool(name="seeds", bufs=4))
        preps = ctx.enter_context(tc.tile_pool(name="preps", bufs=4))
        bigs = ctx.enter_context(tc.tile_pool(name="bigs", bufs=2))
        psums = ctx.enter_context(tc.tile_pool(name="psums", bufs=2, space="PSUM"))
        outs = ctx.enter_context(tc.tile_pool(name="outs", bufs=2))

        wtx_t = singles.tile([Nc, RESOLUTION], F32)
        nc.sync.dma_start(out=wtx_t[:], in_=wtx_d[:])
        wty_t = singles.tile([Nc, RESOLUTION], F32)
        nc.sync.dma_start(out=wty_t[:], in_=wty_d[:])
        sel_t = []
        for q in range(KB):
            st = singles.tile([KB * Nc, Nc], F32, tag=f"sel{q}", name=f"sel{q}")
            nc.sync.dma_start(out=st[:], in_=sel_d[q])
            sel_t.append(st)

        def bias_tile(name, v):
            bt = singles.tile([128, 1], F32, tag=f"bias_{name}", name=f"bias_{name}")
            nc.vector.memset(bt[:], float(v))
            return bt

        seed_bias = {}
        for j in range(njobs):
            for s in range(2):
                seed_bias[(j, s)] = bias_tile(f"s{j}_{s}", -SQNF * float(xc[j * JC + s]))
        r2_bias = {j: bias_tile(f"r{j}", -4 * NF * h * float(centers[j]))
                   for j in range(njobs)}
        zero_bias = bias_tile("z", 0.0)

        raw_t = []
        for dg in range(DG):
            raw = raws.tile([128, CH * 2], F32, tag="raw", name=f"raw{dg}")
            dsrc = diagrams[dg].rearrange("(p c) t -> p (c t)", p=128)
            for si, eng in enumerate((nc.sync, nc.scalar, nc.sync, nc.scalar)):
                sl = slice(si * CH * 2 // 4, (si + 1) * CH * 2 // 4)
                eng.dma_start(out=raw[:, sl], in_=dsrc[:, sl])
            raw_t.append(raw)

        # ---- Phase 1: all Derivative_Erf seeds (one act table) ----
        # seedbuf[dg]: [128, CH, 4, 2]  (4 = job*2 + s, 2 = (x,y))
        seed_t = []
        for dg in range(DG):
            sb = seeds.tile([128, CH, 4, 2], BF16, tag="seed", name=f"seed{dg}")
            rawp = raw_t[dg].rearrange("p (c t) -> p c t", t=2)  # [128, CH, 2]
            for j in range(njobs):
                for s in range(2):
                    nc.scalar.activation(
                        out=sb[:, :, 2 * j + s], in_=rawp[:, :, :],
                        func=DERF, scale=SQNF, bias=seed_bias[(j, s)][:],
                    )
            seed_t.append(sb)

        # ---- Phase 2: exp ratios + abs pers (second act table) ----
        # then vector chains + matmuls per diagram
        for dg in range(DG):
            rawp = raw_t[dg].rearrange("p (c t) -> p c t", t=2)
            pers = preps.tile([128, CH, 2], BF16, tag="pers", name=f"pers{dg}")
            # |d - b| duplicated into the (x, y) pair slots: in is (b, d) pairs;
            # compute via Abs of (d - b): need the subtraction first (vector),
            # then Abs dup. pd kept fp32.
            pd = preps.tile([128, CH], F32, tag="pd", name=f"pd{dg}")
            nc.vector.tensor_sub(pd[:], rawp[:, :, 1], rawp[:, :, 0])
            nc.scalar.activation(
                out=pers[:, :, :], in_=bcast(pd[:], 2, 2),
                func=ABS, scale=1.0, bias=zero_bias[:],
            )
            r2p = []
            for j in range(njobs):
                r2 = preps.tile([128, CH, 2], BF16, tag=f"r2_{j}", name=f"r2_{j}_{dg}")
                nc.scalar.activation(
                    out=r2[:, :, :], in_=rawp[:, :, :],
                    func=EXP, scale=float(4 * NF * h),
                    bias=r2_bias[j][:],
                )
                r2p.append(r2)

            # +1 padding channel: the matmul stationary reads 32 stride-2
            # elements per channel (PE rows padded to 32-partition strips for
            # PSUM access alignment); the overread spills into the next
            # channel and, for the last one, into this zeroed pad.
            T = bigs.tile([128, CH + 1, Nc, 2], BF16, tag="T", name=f"T{dg}")
            nc.vector.memset(T[:, CH], 0.0)
            sb = seed_t[dg]
            for j in range(njobs):
                j0 = j * JC
                # w-fold: T[:, :, j0:j0+2, :] = seeds * |d-b| (both axes)
                nc.vector.tensor_mul(
                    T[:, :CH, j0:j0 + 2, :],
                    sb[:, :, 2 * j:2 * j + 2, :],
                    bcast(pers[:], 2, 2),
                )
                # sequential chain: cols (k, k+1) = cols (k-2, k-1) * r2
                for k in range(2, JC, 2):
                    nc.vector.tensor_mul(
                        T[:, :CH, j0 + k:j0 + k + 2, :],
                        T[:, :CH, j0 + k - 2:j0 + k, :],
                        bcast(r2p[j][:], 2, 2),
                    )

            hp = psums.tile([KB * Nc, KB * Nc], F32, tag="H", name=f"H{dg}")
            Tx = T[:, :, :, 0]   # [128, CH+1, Nc] stride-2 inner
            Ty = T[:, :, :, 1]
            for g in range(NB):
                # x/y elems of KB consecutive channels each form one uniform
                # stride-2 progression (single free dim, as the ISA requires)
                xs = Tx[:, KB * g:KB * (g + 1), :]
                xs1 = bass.AP(tensor=xs.tensor, offset=xs.offset,
                              ap=[xs.ap[0], [2, KB * Nc]])
                ys = Ty[:, KB * g:KB * (g + 1), :]
                ys1 = bass.AP(tensor=ys.tensor, offset=ys.offset,
                              ap=[ys.ap[0], [2, KB * Nc]])
                nc.tensor.matmul(
                    hp[:],
                    xs1,
                    ys1,
                    start=(g == 0), stop=(g == NB - 1),
                )

            # tail: engines need 32-aligned partition bases, so pull the KB
            # diagonal blocks via selector matmuls: hs = sum_q Sq^T @ H[:, qblk]
            hc = outs.tile([KB * Nc, KB * Nc], F32, tag="hc", name=f"hc{dg}")
            nc.vector.tensor_copy(hc[:], hp[:])
            hps = psums.tile([Nc, Nc], F32, tag="hps", name=f"hps{dg}")
            for q in range(KB):
                nc.tensor.matmul(
                    hps[:], sel_t[q][:], hc[:, Nc * q:Nc * (q + 1)],
                    start=(q == 0), stop=(q == KB - 1),
                )
            hs = outs.tile([Nc, Nc], F32, tag="hs", name=f"hs{dg}")
            nc.vector.tensor_copy(hs[:], hps[:])
            p1 = psums.tile([Nc, RESOLUTION], F32, tag="p1", name=f"p1{dg}")
            nc.tensor.matmul(p1[:], hs[:], wtx_t[:], start=True, stop=True)
            o1 = outs.tile([Nc, RESOLUTION], F32, tag="o1", name=f"o1{dg}")
            nc.vector.tensor_copy(o1[:], p1[:])
            p2 = psums.tile([RESOLUTION, RESOLUTION], F32, tag="p2", name=f"p2{dg}")
            nc.tensor.matmul(p2[:], o1[:], wty_t[:], start=True, stop=True)
            o2 = outs.tile([RESOLUTION, RESOLUTION], F32, tag="o2", name=f"o2{dg}")
            nc.vector.tensor_copy(o2[:], p2[:])
            nc.sync.dma_start(out=out_d[dg], in_=o2[:])

    sel = np.zeros((KB, KB * Nc, Nc), np.float32)
    for q in range(KB):
        sel[q, Nc * q:Nc * (q + 1), :] = np.eye(Nc, dtype=np.float32)
    nc.compile()
    return nc, {"wtx": W.copy(), "wty": W.copy(), "sel": sel}


_CACHE = {}


def _get_built():
    if "k" not in _CACHE:
        _CACHE["k"] = build_kernel(DG=4, N=65536, Nc=20, njobs=2)
    return _CACHE["k"]


def kernel(diagrams):
    """Full-input entry point: diagrams [32, 65536, 2] fp32 -> [32, 50, 50] fp32.

    Shards the batch axis over 8 NeuronCores (4 diagrams each), runs the
    Bass kernel SPMD, gathers per-core outputs.
    """
    from concourse.bass_utils import run_bass_kernel_spmd

    diagrams = np.ascontiguousarray(np.asarray(diagrams, dtype=np.float32))
    B, N, two = diagrams.shape
    assert (B, N, two) == (32, 65536, 2), (B, N, two)
    nc, consts = _get_built()
    in_maps = []
    for core in range(8):
        m = {"diagrams": diagrams[core * 4:(core + 1) * 4]}
        m.update(consts)
        in_maps.append(m)
    res = run_bass_kernel_spmd(nc, in_maps, core_ids=list(range(8)))
    out = np.concatenate([res.results[c]["out"] for c in range(8)], axis=0)
    return out.astype(np.float32)
